# revision 1
# baseline (speedup 1.0000x reference)
"""Trainium2 Bass kernel for blocksparse (sink+local) Llama attention.

Sharding: tensor-parallel by head across 8 NeuronCores. Core c computes
q-heads [4c, 4c+4) and kv-head c (the matching GQA group):
  - q/k/v projections column-parallel (per-core weight slices)
  - RoPE + blocksparse streaming attention fully head-local
  - o_proj row-parallel: each core emits a partial [S, HID] product
The row-parallel all-reduce is done at unshard time on the host (an 8-way
fp32 sum), which is far cheaper than an on-device collective here.

Attention computes S^T = K_blk^T Q_blk per granted block (both operands in
the natural [d, s] projection layout, so no transposes anywhere): exp(S^T)
lands directly in SBUF as P^T for the P^T @ V accumulation, softmax column
sums come from an all-ones stationary matmul (which also broadcasts them
across partitions for free), and the 1/sum normalization is fused into the
single DVE op that writes the attention output tile.

Everything on device runs in bf16 with fp32 PSUM accumulation.
"""

import sys

sys.path.insert(0, "/opt/trn_rl_repo")

import math
from contextlib import ExitStack

import ml_dtypes
import numpy as np

import concourse.bass as bass
import concourse.tile as tile
from concourse import bacc, mybir
from concourse.masks import make_lower_triangular

BF16 = mybir.dt.bfloat16
F32 = mybir.dt.float32
NPBF = ml_dtypes.bfloat16

N_CORES = 8
S = 4096
HID = 4096
NH, NKV, D = 32, 8, 128
QH = NH // N_CORES          # 4 q heads per core
BLK = 128
NB = S // BLK               # 32 blocks
LOCAL_NB = 8
SCHUNK = 512                # s-columns processed per phase-1 step
NSC = S // SCHUNK           # 8
HT = HID // 128             # 32 contraction tiles
MASK_VAL = -30000.0
THETA = 10000.0


def _rope_into(nc, pool, dst, ps, cos_c, sin_c, width):
    """dst(bf16) = ps * cos_c + swap_halves(ps) * sin_c  (sin_c sign-baked).

    ps is a [128, width] fp32 PSUM tile holding a projection output d-block;
    partition p is feature dim d. swap_halves pairs d <-> d+64.
    """
    t0 = pool.tile([128, SCHUNK], F32, tag="rope_t0", name="rope_t0")
    t1 = pool.tile([128, SCHUNK], F32, tag="rope_t1", name="rope_t1")
    nc.vector.tensor_mul(t0[:, :width], ps[:, :width], cos_c[:, :width])
    nc.vector.tensor_mul(t1[0:64, :width], ps[64:128, :width], sin_c[0:64, :width])
    nc.vector.tensor_mul(t1[64:128, :width], ps[0:64, :width], sin_c[64:128, :width])
    nc.vector.tensor_add(dst, t0[:, :width], t1[:, :width])


def _emit_body(nc, tc, persist, aps):
    hsT, wq, wk, wv, wo, cos2, sin2, out_p = aps

    qT = persist.tile([128, QH * S], BF16, name="qT")      # [d | (qhead, s)]
    kT = persist.tile([128, S], BF16, name="kT")           # [d | s]
    vN = persist.tile([128, NB * 128], BF16, name="vN")    # [s_in_blk | (blk, d)]
    attnT = persist.tile([128, QH * S], BF16, name="attnT")
    wq_sb = persist.tile([128, HT * QH * 128], BF16, name="wq_sb")
    wk_sb = persist.tile([128, HT * 128], BF16, name="wk_sb")
    wv_sb = persist.tile([128, HT * 128], BF16, name="wv_sb")
    ones = persist.tile([128, 128], BF16, name="ones")
    tri2 = persist.tile([128, 128], F32, name="tri2")      # val strictly-lower (k>q)

    nc.vector.memset(ones, 1.0)
    make_lower_triangular(nc, tri2, val=MASK_VAL, diag=False)

    GRP = 8                      # htiles per grouped DMA

    def _load_wq_group(g):
        # one 3D-AP DMA for 8 htiles of wq: [ht*128+p, c] -> [p, ht*512+c]
        nc.sync.dma_start(
            out=wq_sb[:, g * GRP * 512 : (g + 1) * GRP * 512].rearrange(
                "p (t c) -> p t c", c=512
            ),
            in_=wq[g * GRP * 128 : (g + 1) * GRP * 128, :].rearrange(
                "(t p) c -> p t c", p=128
            ),
        )

    # ---------------- phase 1: projections + RoPE ----------------
    ph1 = ExitStack()
    hs_pool = ph1.enter_context(tc.tile_pool(name="hs_pool", bufs=5))
    tab_pool = ph1.enter_context(tc.tile_pool(name="tab_pool", bufs=2))
    rope_pool = ph1.enter_context(tc.tile_pool(name="rope_pool", bufs=3))
    ps_q = ph1.enter_context(tc.tile_pool(name="ps_q", bufs=5, space="PSUM"))
    ps_v = ph1.enter_context(tc.tile_pool(name="ps_v", bufs=2, space="PSUM"))

    for sc in range(NSC):
        scol = slice(sc * SCHUNK, (sc + 1) * SCHUNK)
        cos_c = tab_pool.tile([128, SCHUNK], F32, tag="cos_c", name="cos_c")
        sin_c = tab_pool.tile([128, SCHUNK], F32, tag="sin_c", name="sin_c")
        hs_grps = []
        for g in range(HT // GRP):
            if sc == 0:
                # interleave weight group loads with the first hs chunk so
                # the first matmuls aren't stuck behind the weight prefetch
                _load_wq_group(g)
            hg = hs_pool.tile([128, GRP * SCHUNK], BF16, tag="hs",
                              name=f"hs_{sc}_{g}")
            nc.sync.dma_start(
                out=hg.rearrange("p (t c) -> p t c", c=SCHUNK),
                in_=hsT[g * GRP * 128 : (g + 1) * GRP * 128, scol].rearrange(
                    "(t p) c -> p t c", p=128
                ),
            )
            hs_grps.append(hg)
        if sc == 0:
            nc.sync.dma_start(
                out=wk_sb.rearrange("p (t c) -> p t c", c=128),
                in_=wk.rearrange("(t p) c -> p t c", p=128),
            )
            nc.sync.dma_start(
                out=wv_sb.rearrange("p (t c) -> p t c", c=128),
                in_=wv.rearrange("(t p) c -> p t c", p=128),
            )

        def _hs(ht):
            return hs_grps[ht // GRP][:, (ht % GRP) * SCHUNK : (ht % GRP + 1) * SCHUNK]
        # tables are consumed by RoPE at chunk end; keep them out of the
        # DMA queue's critical head during the compute-feeding loads
        nc.sync.dma_start(out=cos_c, in_=cos2[:, scol])
        nc.sync.dma_start(out=sin_c, in_=sin2[:, scol])
        # q (4 head d-blocks) + k projections, transposed layout [d, s].
        # For the first chunk go ht-outer with all 5 psums live so PE has
        # ~1.1us of work per arriving hs tile (DMA-matched at startup);
        # later chunks are fully prefetched and use the output-outer order.
        if sc == 0:
            psqs = [
                ps_q.tile([128, SCHUNK], F32, tag="ps_q", name=f"ps_qp{db}")
                for db in range(QH)
            ]
            psk = ps_q.tile([128, SCHUNK], F32, tag="ps_q", name="ps_kp")
            for ht in range(HT):
                for db in range(QH):
                    nc.tensor.matmul(
                        psqs[db],
                        lhsT=wq_sb[:, ht * 512 + db * 128 : ht * 512 + (db + 1) * 128],
                        rhs=_hs(ht),
                        start=(ht == 0),
                        stop=(ht == HT - 1),
                    )
                nc.tensor.matmul(
                    psk,
                    lhsT=wk_sb[:, ht * 128 : (ht + 1) * 128],
                    rhs=_hs(ht),
                    start=(ht == 0),
                    stop=(ht == HT - 1),
                )
            for db in range(QH):
                _rope_into(
                    nc, rope_pool,
                    qT[:, db * S + sc * SCHUNK : db * S + (sc + 1) * SCHUNK],
                    psqs[db], cos_c, sin_c, SCHUNK,
                )
            _rope_into(nc, rope_pool, kT[:, scol], psk, cos_c, sin_c, SCHUNK)
        else:
            for db in range(QH):
                ps = ps_q.tile([128, SCHUNK], F32, tag="ps_q", name="ps_qp")
                for ht in range(HT):
                    nc.tensor.matmul(
                        ps,
                        lhsT=wq_sb[:, ht * 512 + db * 128 : ht * 512 + (db + 1) * 128],
                        rhs=_hs(ht),
                        start=(ht == 0),
                        stop=(ht == HT - 1),
                    )
                _rope_into(
                    nc, rope_pool,
                    qT[:, db * S + sc * SCHUNK : db * S + (sc + 1) * SCHUNK],
                    ps, cos_c, sin_c, SCHUNK,
                )
            psk = ps_q.tile([128, SCHUNK], F32, tag="ps_q", name="ps_kp")
            for ht in range(HT):
                nc.tensor.matmul(
                    psk,
                    lhsT=wk_sb[:, ht * 128 : (ht + 1) * 128],
                    rhs=_hs(ht),
                    start=(ht == 0),
                    stop=(ht == HT - 1),
                )
            _rope_into(nc, rope_pool, kT[:, scol], psk, cos_c, sin_c, SCHUNK)
        # v projection, natural layout [s, d] (no RoPE)
        for sb in range(SCHUNK // 128):
            g = sc * (SCHUNK // 128) + sb
            psv = ps_v.tile([128, 128], F32, tag="ps_v", name="ps_vp")
            for ht in range(HT):
                nc.tensor.matmul(
                    psv,
                    lhsT=_hs(ht)[:, sb * 128 : (sb + 1) * 128],
                    rhs=wv_sb[:, ht * 128 : (ht + 1) * 128],
                    start=(ht == 0),
                    stop=(ht == HT - 1),
                )
            nc.scalar.copy(out=vN[:, g * 128 : (g + 1) * 128], in_=psv)
    ph1.close()

    # -------- phase 2+3 merged: attention with o_proj software-pipelined --------
    # o_proj for block i-1 is emitted between each attention iteration's
    # S^T/exp and ones/PV steps: the in-order PE runs these ready matmuls
    # while ACT computes exp, instead of stalling on the softmax chain.
    ph2 = ExitStack()
    wo_pool = ph2.enter_context(tc.tile_pool(name="wo_pool", bufs=1))
    pt_pool = ph2.enter_context(tc.tile_pool(name="pt_pool", bufs=4))
    rb_pool = ph2.enter_context(tc.tile_pool(name="rb_pool", bufs=3))
    out_pool = ph2.enter_context(tc.tile_pool(name="out_pool", bufs=6))
    ps_S = ph2.enter_context(tc.tile_pool(name="ps_S", bufs=1, space="PSUM"))
    ps_O = ph2.enter_context(tc.tile_pool(name="ps_O", bufs=1, space="PSUM"))
    ps_sum = ph2.enter_context(tc.tile_pool(name="ps_sum", bufs=1, space="PSUM"))
    ps_o3 = ph2.enter_context(tc.tile_pool(name="ps_o3", bufs=3, space="PSUM"))

    wo_sb = wo_pool.tile([128, QH * HID], BF16, name="wo_sb")
    nc.sync.dma_start(
        out=wo_sb.rearrange("p (t c) -> p t c", c=HID),
        in_=wo.rearrange("(t p) c -> p t c", p=128),
    )

    def _oproj_group(sb, ct):
        ps = ps_o3.tile([128, 512], F32, tag="ps3", name="ps3")
        for ht in range(QH):
            nc.tensor.matmul(
                ps,
                lhsT=attnT[:, ht * S + sb * 128 : ht * S + (sb + 1) * 128],
                rhs=wo_sb[:, ht * HID + ct * 512 : ht * HID + (ct + 1) * 512],
                start=(ht == 0),
                stop=(ht == QH - 1),
            )
        ob = out_pool.tile([128, 512], BF16, tag="ob", name="ob")
        nc.vector.tensor_copy(ob, ps)
        nc.sync.dma_start(
            out=out_p[sb * 128 : (sb + 1) * 128, ct * 512 : (ct + 1) * 512],
            in_=ob,
        )

    for i in range(NB):
        L = min(i, LOCAL_NB)       # number of local blocks
        js = i - L + 1             # first local block (>= 1 when L > 0)
        W = (L + 1) * 128
        blocks = list(range(js, i + 1)) if i >= 1 else []
        blocks.append(0)           # sink block last in col layout
        nblk = len(blocks)
        diag_off = (L - 1) * 128 if i >= 1 else 0
        for hq in range(QH):
            qs = qT[:, hq * S + i * 128 : hq * S + (i + 1) * 128]
            # S^T[k, q] per granted block, side by side in one PSUM tile
            S_ps = ps_S.tile([128, 1152], F32, tag="S", name="S_ps")
            for bi, j in enumerate(blocks):
                nc.tensor.matmul(
                    S_ps[:, bi * 128 : (bi + 1) * 128],
                    lhsT=kT[:, j * 128 : (j + 1) * 128],
                    rhs=qs,
                    start=True,
                    stop=True,
                )
            # token-causal mask on the diagonal block (k > q strictly-lower)
            nc.vector.tensor_add(
                S_ps[:, diag_off : diag_off + 128],
                S_ps[:, diag_off : diag_off + 128],
                tri2,
            )
            PT = pt_pool.tile([128, 1152], BF16, tag="PT", name="PT")
            # exp per PSUM bank so ones/PV can consume early blocks while
            # ACT still processes later ones
            for b0 in range(0, W, 512):
                b1 = min(b0 + 512, W)
                nc.scalar.activation(
                    out=PT[:, b0:b1],
                    in_=S_ps[:, b0:b1],
                    func=mybir.ActivationFunctionType.Exp,
                )
            # PE filler while exp runs: o_proj for the previous block
            if i >= 1:
                _oproj_group(i - 1, 2 * hq)
                _oproj_group(i - 1, 2 * hq + 1)
            # column sums broadcast via all-ones stationary; P^T @ V alongside
            sum_ps = ps_sum.tile([128, 128], F32, tag="sum", name="sum_ps")
            O_ps = ps_O.tile([128, 128], F32, tag="O", name="O_ps")
            for bi, j in enumerate(blocks):
                pts = PT[:, bi * 128 : (bi + 1) * 128]
                nc.tensor.matmul(
                    sum_ps, lhsT=ones, rhs=pts,
                    start=(bi == 0), stop=(bi == nblk - 1),
                )
                nc.tensor.matmul(
                    O_ps, lhsT=vN[:, j * 128 : (j + 1) * 128], rhs=pts,
                    start=(bi == 0), stop=(bi == nblk - 1),
                )
            sum_sb = rb_pool.tile([128, 128], F32, tag="sum_sb", name="sum_sb")
            nc.vector.tensor_copy(sum_sb, sum_ps)
            rb = rb_pool.tile([128, 128], F32, tag="rb", name="rb")
            nc.vector.reciprocal(rb, sum_sb)
            nc.vector.tensor_mul(
                attnT[:, hq * S + i * 128 : hq * S + (i + 1) * 128], O_ps, rb
            )
    for hq in range(QH):
        _oproj_group(NB - 1, 2 * hq)
        _oproj_group(NB - 1, 2 * hq + 1)
    ph2.close()


def build_kernel(nc, reps=1):
    hsT = nc.dram_tensor("hsT", [HID, S], BF16, kind="ExternalInput").ap()
    wq = nc.dram_tensor("wq", [HID, QH * D], BF16, kind="ExternalInput").ap()
    wk = nc.dram_tensor("wk", [HID, D], BF16, kind="ExternalInput").ap()
    wv = nc.dram_tensor("wv", [HID, D], BF16, kind="ExternalInput").ap()
    wo = nc.dram_tensor("wo", [QH * D, HID], BF16, kind="ExternalInput").ap()
    cos2 = nc.dram_tensor("cos2", [128, S], F32, kind="ExternalInput").ap()
    sin2 = nc.dram_tensor("sin2", [128, S], F32, kind="ExternalInput").ap()
    out_p = nc.dram_tensor("out_p", [S, HID], BF16, kind="ExternalOutput").ap()
    aps = (hsT, wq, wk, wv, wo, cos2, sin2, out_p)

    with tile.TileContext(nc) as tc:
        with tc.tile_pool(name="persist", bufs=1) as persist:
            for _rep in range(reps):
                _emit_body(nc, tc, persist, aps)
    return nc


_NC = {}


def _get_nc(reps=1):
    if reps not in _NC:
        nc = bacc.Bacc(
            "TRN2", target_bir_lowering=False, debug=False, num_devices=N_CORES
        )
        build_kernel(nc, reps=reps)
        nc.compile()
        _NC[reps] = nc
    return _NC[reps]


def make_exec_fn(nc, n_cores=N_CORES):
    """Build a reusable sharded executor for a compiled Bass module.

    Mirrors bass2jax.run_bass_via_pjrt's multi-core branch, but without
    donation so the zero output buffers can stay device-resident across
    repeated calls (for benchmarking).
    """
    import jax
    from jax.sharding import Mesh, NamedSharding, PartitionSpec
    from jax.experimental.shard_map import shard_map

    from concourse import bass2jax

    bass2jax.install_neuronx_cc_hook()

    partition_name = nc.partition_id_tensor.name if nc.partition_id_tensor else None
    in_names, out_names, out_avals, zero_outs = [], [], [], []
    for alloc in nc.m.functions[0].allocations:
        if not isinstance(alloc, mybir.MemoryLocationSet):
            continue
        name = alloc.memorylocations[0].name
        if alloc.kind == "ExternalInput":
            if name != partition_name:
                in_names.append(name)
        elif alloc.kind == "ExternalOutput":
            out_names.append(name)
            shape = tuple(alloc.tensor_shape)
            dtype = mybir.dt.np(alloc.dtype)
            out_avals.append(jax.core.ShapedArray(shape, dtype))
            zero_outs.append(np.zeros(shape, dtype))
    all_in_names = list(in_names) + list(out_names)
    if partition_name is not None:
        all_in_names.append(partition_name)
    all_in_names = tuple(all_in_names)

    def _body(*args):
        operands = list(args)
        if partition_name is not None:
            operands.append(bass2jax.partition_id_tensor())
        outs = bass2jax._bass_exec_p.bind(
            *operands,
            out_avals=tuple(out_avals),
            in_names=all_in_names,
            out_names=tuple(out_names),
            lowering_input_output_aliases=(),
            sim_require_finite=True,
            sim_require_nnan=True,
            nc=nc,
        )
        return tuple(outs)

    devices = jax.devices()[:n_cores]
    mesh = Mesh(np.asarray(devices), ("core",))
    spec = PartitionSpec("core")
    in_specs = (spec,) * (len(in_names) + len(out_names))
    out_specs = (spec,) * len(out_names)
    fn = jax.jit(
        shard_map(
            _body, mesh=mesh, in_specs=in_specs, out_specs=out_specs, check_rep=False
        ),
        keep_unused=True,
    )
    return fn, in_names, out_names, zero_outs, NamedSharding(mesh, spec)


_EXEC = None


def _get_exec():
    global _EXEC
    if _EXEC is None:
        _EXEC = make_exec_fn(_get_nc())
    return _EXEC


def _concat_args(in_maps, in_names, zero_outs):
    concat_in = [
        np.concatenate([np.asarray(in_maps[c][nm]) for c in range(N_CORES)], axis=0)
        for nm in in_names
    ]
    concat_zeros = [
        np.zeros((N_CORES * z.shape[0], *z.shape[1:]), z.dtype) for z in zero_outs
    ]
    return concat_in + concat_zeros


def _host_inputs(hidden_states, wq, wk, wv, wo):
    hs = np.asarray(hidden_states, np.float32).reshape(S, HID)
    hsT = np.ascontiguousarray(hs.T).astype(NPBF)

    scale = 1.0 / math.sqrt(D)
    inv_freq = 1.0 / (THETA ** (np.arange(0, D, 2, dtype=np.float32) / D))
    t = np.arange(S, dtype=np.float32)
    freqs = np.outer(t, inv_freq)                      # [S, 64]
    cosT = np.cos(freqs).T.astype(np.float32)          # [64, S]
    sinT = np.sin(freqs).T.astype(np.float32)
    cos2 = np.ascontiguousarray(np.concatenate([cosT, cosT], 0))   # [128, S]
    sin2 = np.ascontiguousarray(np.concatenate([-sinT, sinT], 0))  # [128, S]

    wq = np.asarray(wq, np.float32) * scale
    in_maps = []
    for c in range(N_CORES):
        in_maps.append(
            {
                "hsT": hsT,
                "wq": np.ascontiguousarray(wq[:, c * 512 : (c + 1) * 512]).astype(NPBF),
                "wk": np.ascontiguousarray(
                    np.asarray(wk, np.float32)[:, c * 128 : (c + 1) * 128]
                ).astype(NPBF),
                "wv": np.ascontiguousarray(
                    np.asarray(wv, np.float32)[:, c * 128 : (c + 1) * 128]
                ).astype(NPBF),
                "wo": np.ascontiguousarray(
                    np.asarray(wo, np.float32)[c * 512 : (c + 1) * 512, :]
                ).astype(NPBF),
                "cos2": cos2,
                "sin2": sin2,
            }
        )
    return in_maps


def _reduce_out(out_concat):
    acc = (
        np.asarray(out_concat)
        .reshape(N_CORES, S, HID)
        .astype(np.float32)
        .sum(axis=0)
    )
    return np.ascontiguousarray(acc).reshape(1, S, HID)


def run(hidden_states, wq, wk, wv, wo):
    """Returns full fp32 output [1, S, HID]."""
    import jax

    fn, in_names, out_names, zero_outs, sh = _get_exec()
    in_maps = _host_inputs(hidden_states, wq, wk, wv, wo)
    args = _concat_args(in_maps, in_names, zero_outs)
    outs = jax.block_until_ready(fn(*args))
    return _reduce_out(outs[0])


def bench(hidden_states, wq, wk, wv, wo, iters=10):
    """Repeated device-resident executions; returns (out, per-iter seconds)."""
    import time

    import jax

    fn, in_names, out_names, zero_outs, sh = _get_exec()
    in_maps = _host_inputs(hidden_states, wq, wk, wv, wo)
    args = _concat_args(in_maps, in_names, zero_outs)
    dev_args = jax.block_until_ready([jax.device_put(a, sh) for a in args])
    outs = jax.block_until_ready(fn(*dev_args))  # warm-up + compile
    times = []
    for _ in range(iters):
        t0 = time.perf_counter()
        o = fn(*dev_args)
        jax.block_until_ready(o)
        times.append(time.perf_counter() - t0)
    # async-queued: submit all, block once — measures pipelined dispatch
    for n in (1, iters):
        t0 = time.perf_counter()
        os_ = [fn(*dev_args) for _ in range(n)]
        jax.block_until_ready(os_)
        times.append((time.perf_counter() - t0) / n)
    return _reduce_out(outs[0]), times


def kernel(hidden_states, wq, wk, wv, wo):
    return run(hidden_states, wq, wk, wv, wo)

